# revision 1
# baseline (speedup 1.0000x reference)
"""Trainium2 Bass kernel for the NeuralFVSolver problem.

Strategy: pure data parallel over batch (16 batches -> 8 cores x 2).
Per core, the 63 autoregressive steps run fully unrolled. Per step:
  - shock detection is_shock[j] = state[j] > state[j-1]  (the reference's
    char_L > s_rh > char_R condition algebraically reduces to rR > rL)
  - min-distance-to-shock via two min-plus tensor_tensor_scan passes
    (1D distance transform; far field saturates at 1.0 where
    exp(-D/sigma) == 0 in fp32 anyway, matching the reference's 1e6 mask)
  - prox = exp(-D/sigma) on the scalar engine
  - stencil features built with sliding-window DMAs into [14,1024] tiles
  - 4-layer MLP as block-diagonal matmuls processing both batch rows in
    one pass (K=28/128); char_speed and dt channels folded into W0/biases
    host-side; exact-erf Gelu with fused bias on the scalar engine
  - state update + clip on the vector engine
"""

import numpy as np
from contextlib import ExitStack

import concourse.bass as bass
import concourse.bacc as bacc
import concourse.tile as tile
from concourse import mybir
from concourse.bass_utils import run_bass_kernel_spmd

F32 = mybir.dt.float32
OP = mybir.AluOpType
AF = mybir.ActivationFunctionType

B, NT, NX = 16, 64, 1024
NSTEP = NT - 1
NCORES = 8
BPC = B // NCORES          # batches per core = 2
KHW = 3                    # stencil half width
S = 2 * KHW + 1            # 7
DX = 0.02
SIGMA = 0.05
HID = 64
PAD = NX + 2 * KHW         # 1030
CHUNK = 512                # matmul moving-dim chunk (fp32 PSUM bank limit)

# exact-cancellation constants for init = (m * C1) + C_BIG
C_BIG = 1.0
C1 = float(np.float32(np.float32(0.5 * DX) - np.float32(C_BIG)))

_compiled = None


def _build_module():
    nc = bacc.Bacc("TRN2", target_bir_lowering=False, debug=False)

    # ---- DRAM I/O ----
    d_state0 = nc.dram_tensor("state0", [BPC, PAD], F32, kind="ExternalInput").ap()
    d_l0v = nc.dram_tensor("l0v", [2 * S, 2 * HID], F32, kind="ExternalInput").ap()
    d_l0p = nc.dram_tensor("l0p", [2 * S, 2 * HID], F32, kind="ExternalInput").ap()
    d_l1 = nc.dram_tensor("l1", [2 * HID, 2 * HID], F32, kind="ExternalInput").ap()
    d_l2 = nc.dram_tensor("l2", [2 * HID, 2 * HID], F32, kind="ExternalInput").ap()
    d_l3 = nc.dram_tensor("l3", [2 * HID, BPC], F32, kind="ExternalInput").ap()
    d_b0 = nc.dram_tensor("b0d", [2 * HID, 1], F32, kind="ExternalInput").ap()
    d_b1 = nc.dram_tensor("b1d", [2 * HID, 1], F32, kind="ExternalInput").ap()
    d_b2 = nc.dram_tensor("b2d", [2 * HID, 1], F32, kind="ExternalInput").ap()
    d_b3 = nc.dram_tensor("b3d", [BPC, 1], F32, kind="ExternalInput").ap()
    d_out = nc.dram_tensor("out", [BPC, NSTEP, NX], F32, kind="ExternalOutput").ap()

    with tile.TileContext(nc) as tc, ExitStack() as ctx:
        pool = ctx.enter_context(tc.tile_pool(name="sb", bufs=1))
        psum = ctx.enter_context(tc.tile_pool(name="ps", bufs=1, space="PSUM"))

        # ---- persistent tiles ----
        p0 = pool.tile([BPC, PAD], F32, tag="p0")
        p1 = pool.tile([BPC, PAD], F32, tag="p1")
        q = pool.tile([BPC, PAD], F32, tag="q")
        sh = pool.tile([BPC, NX + 1], F32, tag="sh")
        m = pool.tile([BPC, NX], F32, tag="m")
        ini = pool.tile([BPC, NX], F32, tag="ini")
        Df = pool.tile([BPC, NX], F32, tag="Df")
        Dd = pool.tile([BPC, NX], F32, tag="Dd")
        tmp = pool.tile([BPC, NX], F32, tag="tmp")
        dxc = pool.tile([BPC, NX], F32, tag="dxc")
        rhs_v = pool.tile([2 * S, NX], F32, tag="rhs_v")
        rhs_p = pool.tile([2 * S, NX], F32, tag="rhs_p")
        a0 = pool.tile([2 * HID, NX], F32, tag="a0")
        a1 = pool.tile([2 * HID, NX], F32, tag="a1")
        a2 = pool.tile([2 * HID, NX], F32, tag="a2")

        l0v = pool.tile([2 * S, 2 * HID], F32, tag="l0v")
        l0p = pool.tile([2 * S, 2 * HID], F32, tag="l0p")
        l1 = pool.tile([2 * HID, 2 * HID], F32, tag="l1")
        l2 = pool.tile([2 * HID, 2 * HID], F32, tag="l2")
        l3 = pool.tile([2 * HID, BPC], F32, tag="l3")
        b0t = pool.tile([2 * HID, 1], F32, tag="b0t")
        b1t = pool.tile([2 * HID, 1], F32, tag="b1t")
        b2t = pool.tile([2 * HID, 1], F32, tag="b2t")
        b3t = pool.tile([BPC, 1], F32, tag="b3t")

        h0 = psum.tile([2 * HID, NX], F32, tag="h0")
        h1 = psum.tile([2 * HID, NX], F32, tag="h1")
        h2 = psum.tile([2 * HID, NX], F32, tag="h2")
        upd = psum.tile([BPC, NX], F32, tag="upd")

        # ---- one-time loads / inits ----
        nc.sync.dma_start(p0[:], d_state0[:])
        for t_, d_ in ((l0v, d_l0v), (l0p, d_l0p), (l1, d_l1), (l2, d_l2),
                       (l3, d_l3), (b0t, d_b0), (b1t, d_b1), (b2t, d_b2),
                       (b3t, d_b3)):
            nc.sync.dma_start(t_[:], d_[:])
        nc.vector.memset(dxc[:], DX)
        nc.vector.memset(sh[:, NX:NX + 1], 0.0)

        def sliding_src(ptile):
            # [BPC, S, NX] overlapping-window view of a [BPC, PAD] tile
            src = ptile[:, 0:S]
            fancy = src.copy()
            fancy.ap = src.ap + [[1, NX]]
            return fancy

        for t in range(NSTEP):
            p_prev = p0 if t % 2 == 0 else p1
            p_cur = p1 if t % 2 == 0 else p0
            st = p_prev[:, KHW:KHW + NX]          # state cells view

            # ---- stencil values DMA (state was ready at end of prev step)
            nc.sync.dma_start(rhs_v[:], sliding_src(p_prev))

            # ---- shock proximity chain (vector engine) ----
            nc.vector.tensor_tensor(sh[:, 0:NX], st, p_prev[:, KHW - 1:KHW - 1 + NX],
                                    OP.is_gt)
            nc.vector.tensor_tensor(m[:], sh[:, 0:NX], sh[:, 1:NX + 1], OP.max)
            nc.vector.tensor_scalar(ini[:], m[:], C1, C_BIG, OP.mult, OP.add)
            nc.vector.tensor_tensor_scan(Df[:], dxc[:], ini[:], C_BIG,
                                         OP.add, OP.min)
            nc.vector.tensor_tensor_scan(Dd[:, ::-1], dxc[:], Df[:, ::-1], C_BIG,
                                         OP.add, OP.min)
            # prox into padded q + replicate edges
            nc.scalar.activation(q[:, KHW:KHW + NX], Dd[:], AF.Exp,
                                 scale=-1.0 / SIGMA)
            nc.vector.tensor_copy(q[:, 0:KHW],
                                  q[:, KHW:KHW + 1].broadcast_to([BPC, KHW]))
            nc.vector.tensor_copy(q[:, KHW + NX:PAD],
                                  q[:, KHW + NX - 1:KHW + NX].broadcast_to([BPC, KHW]))
            nc.sync.dma_start(rhs_p[:], sliding_src(q))

            # ---- MLP (block-diagonal, both batches per pass) ----
            for c0 in range(0, NX, CHUNK):
                cs = slice(c0, c0 + CHUNK)
                nc.tensor.matmul(h0[:, cs], l0v[:], rhs_v[:, cs],
                                 start=True, stop=False)
                nc.tensor.matmul(h0[:, cs], l0p[:], rhs_p[:, cs],
                                 start=False, stop=True)
                nc.scalar.activation(a0[:, cs], h0[:, cs], AF.Gelu, bias=b0t[:])
                nc.tensor.matmul(h1[:, cs], l1[:], a0[:, cs], start=True, stop=True)
                nc.scalar.activation(a1[:, cs], h1[:, cs], AF.Gelu, bias=b1t[:])
                nc.tensor.matmul(h2[:, cs], l2[:], a1[:, cs], start=True, stop=True)
                nc.scalar.activation(a2[:, cs], h2[:, cs], AF.Gelu, bias=b2t[:])
                nc.tensor.matmul(upd[:, cs], l3[:], a2[:, cs], start=True, stop=True)

            # ---- state update + clip into p_cur ----
            nc.vector.scalar_tensor_tensor(tmp[:], upd[:], b3t[:], st,
                                           OP.add, OP.add)
            nc.vector.tensor_scalar(p_cur[:, KHW:KHW + NX], tmp[:], 0.0, 1.0,
                                    OP.max, OP.min)
            nc.vector.tensor_copy(p_cur[:, 0:KHW],
                                  p_cur[:, KHW:KHW + 1].broadcast_to([BPC, KHW]))
            nc.vector.tensor_copy(p_cur[:, KHW + NX:PAD],
                                  p_cur[:, KHW + NX - 1:KHW + NX]
                                  .broadcast_to([BPC, KHW]))

            nc.sync.dma_start(d_out[:, t, :], p_cur[:, KHW:KHW + NX])

    nc.compile()
    return nc


def _prepare_core_inputs(grid_input, dt, W0, b0, W1, b1, W2, b2, W3, b3):
    """Host-side constant folding; returns list of per-core input dicts."""
    f = np.float32
    W0 = W0.astype(f); W1 = W1.astype(f); W2 = W2.astype(f); W3 = W3.astype(f)
    W0v = W0[:, 0:S] - 2.0 * W0[:, S:2 * S]          # vals + folded char_speeds
    W0p = W0[:, 2 * S:3 * S]                          # prox columns
    bd = np.zeros((2, S, 2 * HID), f)                 # block-diag helper

    def blockdiag(Wsub):  # Wsub [HID, k] -> [2k, 2HID]
        k = Wsub.shape[1]
        out = np.zeros((2 * k, 2 * HID), f)
        out[0:k, 0:HID] = Wsub.T
        out[k:2 * k, HID:2 * HID] = Wsub.T
        return out

    l0v = blockdiag(W0v)
    l0p = blockdiag(W0p)
    l1 = blockdiag(W1)
    l2 = blockdiag(W2)

    in_maps = []
    for c in range(NCORES):
        bsel = [BPC * c + i for i in range(BPC)]
        dts = dt[bsel].astype(f)                      # [2]
        s0 = grid_input[bsel, 0, 0, :].astype(f)      # [2, NX]
        s0p = np.concatenate([np.repeat(s0[:, :1], KHW, 1), s0,
                              np.repeat(s0[:, -1:], KHW, 1)], axis=1)  # [2, PAD]
        b0d = np.concatenate([
            (b0 + W0[:, S:2 * S].sum(1) + W0[:, 3 * S] * dts[0]),
            (b0 + W0[:, S:2 * S].sum(1) + W0[:, 3 * S] * dts[1]),
        ]).astype(f)[:, None]
        b1d = np.concatenate([b1, b1]).astype(f)[:, None]
        b2d = np.concatenate([b2, b2]).astype(f)[:, None]
        l3 = np.zeros((2 * HID, BPC), f)
        l3[0:HID, 0] = W3[0] * (dts[0] / DX)
        l3[HID:2 * HID, 1] = W3[0] * (dts[1] / DX)
        b3d = np.array([[b3[0] * dts[0] / DX], [b3[0] * dts[1] / DX]], f)
        in_maps.append({
            "state0": s0p, "l0v": l0v, "l0p": l0p, "l1": l1, "l2": l2,
            "l3": l3, "b0d": b0d, "b1d": b1d, "b2d": b2d, "b3d": b3d,
        })
    return in_maps


def kernel(grid_input, dt, W0, b0, W1, b1, W2, b2, W3, b3, _run_kwargs=None):
    global _compiled
    grid_input = np.asarray(grid_input)
    if _compiled is None:
        _compiled = _build_module()
    nc = _compiled
    in_maps = _prepare_core_inputs(grid_input, np.asarray(dt),
                                   np.asarray(W0), np.asarray(b0),
                                   np.asarray(W1), np.asarray(b1),
                                   np.asarray(W2), np.asarray(b2),
                                   np.asarray(W3), np.asarray(b3))
    kw = _run_kwargs or {}
    r = run_bass_kernel_spmd(nc, in_maps, list(range(NCORES)), **kw)
    out = np.empty((B, 1, NT, NX), np.float32)
    out[:, 0, 0, :] = grid_input[:, 0, 0, :]
    for c in range(NCORES):
        out[BPC * c:BPC * (c + 1), 0, 1:, :] = r.results[c]["out"]
    kernel.last_results = r
    return out


# revision 3
# speedup vs baseline: 1.0109x; 1.0109x over previous
"""Trainium2 Bass kernel for the NeuralFVSolver problem.

Strategy: pure data parallel over batch (16 batches -> 8 cores x 2).
Per core, the 63 autoregressive steps run fully unrolled. Per step:
  - shock detection is_shock[j] = state[j] > state[j-1]  (the reference's
    char_L > s_rh > char_R condition algebraically reduces to rR > rL)
  - prox computed directly in exp space: prox[i] = max_j m[j]*alpha^|i-j|
    with alpha = exp(-dx/sigma), via two max-mult tensor_tensor_scan passes
    (1D distance transform in the max-product semiring; far field
    underflows to 0 exactly like the reference's exp(-1e6/sigma))
  - stencil features built with sliding-window DMAs into [14,1024] tiles
  - 4-layer MLP as block-diagonal float32r matmuls processing both batch
    rows in one pass; char_speed and dt channels folded into W0/biases
    host-side; exact-erf Gelu with fused bias on the scalar engine
  - state + b3 pre-accumulated into PSUM by an fp32 identity-matmul at
    step start (off the critical path); final clip on the vector engine
"""

import math
import numpy as np
from contextlib import ExitStack

import concourse.bass as bass
import concourse.bacc as bacc
import concourse.tile as tile
from concourse import mybir
from concourse.bass_utils import run_bass_kernel_spmd

F32 = mybir.dt.float32
F32R = mybir.dt.float32r
OP = mybir.AluOpType
AF = mybir.ActivationFunctionType

B, NT, NX = 16, 64, 1024
NSTEP = NT - 1
NCORES = 8
BPC = B // NCORES          # batches per core = 2
KHW = 3                    # stencil half width
S = 2 * KHW + 1            # 7
DX = 0.02
SIGMA = 0.05
HID = 64
PAD = NX + 2 * KHW         # 1030
CHUNK = 512                # matmul moving-dim chunk (fp32 PSUM bank limit)

ALPHA = float(np.float32(math.exp(-DX / SIGMA)))       # per-cell decay
BASE0 = float(np.float32(math.exp(-0.5 * DX / SIGMA))) # half-cell seed

_compiled = None


def _build_module():
    nc = bacc.Bacc("TRN2", target_bir_lowering=False, debug=False)

    d_state0 = nc.dram_tensor("state0", [BPC, PAD], F32, kind="ExternalInput").ap()
    d_l0v = nc.dram_tensor("l0v", [2 * S, 2 * HID], F32, kind="ExternalInput").ap()
    d_l0p = nc.dram_tensor("l0p", [2 * S, 2 * HID], F32, kind="ExternalInput").ap()
    d_l1 = nc.dram_tensor("l1", [2 * HID, 2 * HID], F32, kind="ExternalInput").ap()
    d_l2 = nc.dram_tensor("l2", [2 * HID, 2 * HID], F32, kind="ExternalInput").ap()
    d_l3 = nc.dram_tensor("l3", [2 * HID, BPC], F32, kind="ExternalInput").ap()
    d_le = nc.dram_tensor("le", [BPC + 1, BPC], F32, kind="ExternalInput").ap()
    d_b0 = nc.dram_tensor("b0d", [2 * HID, 1], F32, kind="ExternalInput").ap()
    d_b1 = nc.dram_tensor("b1d", [2 * HID, 1], F32, kind="ExternalInput").ap()
    d_b2 = nc.dram_tensor("b2d", [2 * HID, 1], F32, kind="ExternalInput").ap()
    d_out = nc.dram_tensor("out", [BPC, NSTEP, NX], F32, kind="ExternalOutput").ap()

    with tile.TileContext(nc) as tc, ExitStack() as ctx:
        pool = ctx.enter_context(tc.tile_pool(name="sb", bufs=1))
        psum = ctx.enter_context(tc.tile_pool(name="ps", bufs=1, space="PSUM"))

        # state-padded tiles (rows 0-1 = state, row 2 = const ones for the
        # identity matmul's bias row)
        p0 = pool.tile([BPC + 1, PAD], F32, tag="p0")
        p1 = pool.tile([BPC + 1, PAD], F32, tag="p1")
        q = pool.tile([BPC, PAD], F32, tag="q")
        sh = pool.tile([BPC, NX + 1], F32, tag="sh")
        m = pool.tile([BPC, NX], F32, tag="m")
        bse = pool.tile([BPC, NX], F32, tag="bse")
        Pf = pool.tile([BPC, NX], F32, tag="Pf")
        alc = pool.tile([BPC, NX], F32, tag="alc")
        rhs_v = pool.tile([2 * S, NX], F32R, tag="rhs_v")
        rhs_p = pool.tile([2 * S, NX], F32R, tag="rhs_p")
        a0 = pool.tile([2 * HID, NX], F32R, tag="a0")
        a1 = pool.tile([2 * HID, NX], F32R, tag="a1")
        a2 = pool.tile([2 * HID, NX], F32R, tag="a2")

        l0v_f = pool.tile([2 * S, 2 * HID], F32, tag="l0v_f")
        l0p_f = pool.tile([2 * S, 2 * HID], F32, tag="l0p_f")
        l1_f = pool.tile([2 * HID, 2 * HID], F32, tag="l1_f")
        l2_f = pool.tile([2 * HID, 2 * HID], F32, tag="l2_f")
        l3_f = pool.tile([2 * HID, BPC], F32, tag="l3_f")
        l0v = pool.tile([2 * S, 2 * HID], F32R, tag="l0v")
        l0p = pool.tile([2 * S, 2 * HID], F32R, tag="l0p")
        l1 = pool.tile([2 * HID, 2 * HID], F32R, tag="l1")
        l2 = pool.tile([2 * HID, 2 * HID], F32R, tag="l2")
        l3 = pool.tile([2 * HID, BPC], F32R, tag="l3")
        le = pool.tile([BPC + 1, BPC], F32, tag="le")
        b0t = pool.tile([2 * HID, 1], F32, tag="b0t")
        b1t = pool.tile([2 * HID, 1], F32, tag="b1t")
        b2t = pool.tile([2 * HID, 1], F32, tag="b2t")

        h0 = psum.tile([2 * HID, NX], F32, tag="h0")
        h1 = psum.tile([2 * HID, NX], F32, tag="h1")
        h2 = psum.tile([2 * HID, NX], F32, tag="h2")
        upd = psum.tile([BPC, NX], F32, tag="upd")

        # ---- one-time loads / inits ----
        # whole-tile memset to 1.0 first (row 2 = the ones row for the
        # identity matmul); the state DMA then overwrites rows 0-1
        nc.vector.memset(p0[:], 1.0)
        nc.vector.memset(p1[:], 1.0)
        nc.sync.dma_start(p0[0:BPC, :], d_state0[:])
        for t_, d_ in ((l0v_f, d_l0v), (l0p_f, d_l0p), (l1_f, d_l1),
                       (l2_f, d_l2), (l3_f, d_l3), (le, d_le),
                       (b0t, d_b0), (b1t, d_b1), (b2t, d_b2)):
            nc.sync.dma_start(t_[:], d_[:])
        for src, dst in ((l0v_f, l0v), (l0p_f, l0p), (l1_f, l1),
                         (l2_f, l2), (l3_f, l3)):
            nc.vector.tensor_copy(dst[:], src[:])
        nc.vector.memset(alc[:], ALPHA)
        nc.vector.memset(sh[:, NX:NX + 1], 0.0)

        def sliding_src(ptile, dt_):
            src = ptile[0:BPC, 0:S]
            fancy = src.copy()
            fancy.ap = src.ap + [[1, NX]]
            return fancy.bitcast(dt_)

        for t in range(NSTEP):
            p_prev = p0 if t % 2 == 0 else p1
            p_cur = p1 if t % 2 == 0 else p0
            st = p_prev[0:BPC, KHW:KHW + NX]

            # state + b3 into PSUM early (identity matmul, fp32, exact)
            for c0 in range(0, NX, CHUNK):
                nc.tensor.matmul(upd[:, c0:c0 + CHUNK], le[:],
                                 p_prev[:, KHW + c0:KHW + c0 + CHUNK],
                                 start=True, stop=False)

            nc.sync.dma_start(rhs_v[:], sliding_src(p_prev, F32R))

            # ---- shock proximity in exp space (vector engine) ----
            nc.vector.tensor_tensor(sh[:, 0:NX], st,
                                    p_prev[0:BPC, KHW - 1:KHW - 1 + NX], OP.is_gt)
            nc.vector.tensor_tensor(m[:], sh[:, 0:NX], sh[:, 1:NX + 1], OP.max)
            nc.vector.tensor_scalar(bse[:], m[:], BASE0, None, OP.mult)
            nc.vector.tensor_tensor_scan(Pf[:], alc[:], bse[:], 0.0,
                                         OP.mult, OP.max)
            nc.vector.tensor_tensor_scan(q[:, KHW:KHW + NX][:, ::-1], alc[:],
                                         Pf[:, ::-1], 0.0, OP.mult, OP.max)
            nc.vector.tensor_copy(q[:, 0:KHW],
                                  q[:, KHW:KHW + 1].broadcast_to([BPC, KHW]))
            nc.vector.tensor_copy(q[:, KHW + NX:PAD],
                                  q[:, KHW + NX - 1:KHW + NX].broadcast_to([BPC, KHW]))
            nc.sync.dma_start(rhs_p[:], sliding_src(q, F32R))

            # ---- MLP ladder, pipelined across the two 512-cell halves ----
            for c0 in range(0, NX, CHUNK):
                cs = slice(c0, c0 + CHUNK)
                nc.tensor.matmul(h0[:, cs], l0v[:], rhs_v[:, cs],
                                 start=True, stop=False)
                nc.tensor.matmul(h0[:, cs], l0p[:], rhs_p[:, cs],
                                 start=False, stop=True)
                nc.scalar.activation(a0[:, cs], h0[:, cs], AF.Gelu, bias=b0t[:])
                nc.tensor.matmul(h1[:, cs], l1[:], a0[:, cs], start=True, stop=True)
                nc.scalar.activation(a1[:, cs], h1[:, cs], AF.Gelu, bias=b1t[:])
                nc.tensor.matmul(h2[:, cs], l2[:], a1[:, cs], start=True, stop=True)
                nc.scalar.activation(a2[:, cs], h2[:, cs], AF.Gelu, bias=b2t[:])
                nc.tensor.matmul(upd[:, cs], l3[:], a2[:, cs],
                                 start=False, stop=True)
                nc.vector.tensor_scalar(p_cur[0:BPC, KHW + c0:KHW + c0 + CHUNK],
                                        upd[:, cs], 0.0, 1.0, OP.max, OP.min)

            nc.vector.tensor_copy(p_cur[0:BPC, 0:KHW],
                                  p_cur[0:BPC, KHW:KHW + 1].broadcast_to([BPC, KHW]))
            nc.vector.tensor_copy(p_cur[0:BPC, KHW + NX:PAD],
                                  p_cur[0:BPC, KHW + NX - 1:KHW + NX]
                                  .broadcast_to([BPC, KHW]))

            nc.sync.dma_start(d_out[:, t, :], p_cur[0:BPC, KHW:KHW + NX])

    nc.compile()
    return nc


def _prepare_core_inputs(grid_input, dt, W0, b0, W1, b1, W2, b2, W3, b3):
    """Host-side constant folding; returns list of per-core input dicts."""
    f = np.float32
    W0 = W0.astype(f); W1 = W1.astype(f); W2 = W2.astype(f); W3 = W3.astype(f)
    W0v = W0[:, 0:S] - 2.0 * W0[:, S:2 * S]          # vals + folded char_speeds
    W0p = W0[:, 2 * S:3 * S]                          # prox columns

    def blockdiag(Wsub):  # Wsub [HID, k] -> [2k, 2HID]
        k = Wsub.shape[1]
        out = np.zeros((2 * k, 2 * HID), f)
        out[0:k, 0:HID] = Wsub.T
        out[k:2 * k, HID:2 * HID] = Wsub.T
        return out

    l0v = blockdiag(W0v)
    l0p = blockdiag(W0p)
    l1b = blockdiag(W1)
    l2b = blockdiag(W2)

    in_maps = []
    for c in range(NCORES):
        bsel = [BPC * c + i for i in range(BPC)]
        dts = dt[bsel].astype(f)
        s0 = grid_input[bsel, 0, 0, :].astype(f)
        s0p = np.concatenate([np.repeat(s0[:, :1], KHW, 1), s0,
                              np.repeat(s0[:, -1:], KHW, 1)], axis=1)
        b0d = np.concatenate([
            (b0 + W0[:, S:2 * S].sum(1) + W0[:, 3 * S] * dts[0]),
            (b0 + W0[:, S:2 * S].sum(1) + W0[:, 3 * S] * dts[1]),
        ]).astype(f)[:, None]
        b1d = np.concatenate([b1, b1]).astype(f)[:, None]
        b2d = np.concatenate([b2, b2]).astype(f)[:, None]
        l3 = np.zeros((2 * HID, BPC), f)
        l3[0:HID, 0] = W3[0] * (dts[0] / DX)
        l3[HID:2 * HID, 1] = W3[0] * (dts[1] / DX)
        le = np.zeros((BPC + 1, BPC), f)   # [I2; b3*dt/dx] state+bias matmul
        le[0, 0] = 1.0
        le[1, 1] = 1.0
        le[2, 0] = b3[0] * dts[0] / DX
        le[2, 1] = b3[0] * dts[1] / DX
        in_maps.append({
            "state0": s0p, "l0v": l0v, "l0p": l0p, "l1": l1b, "l2": l2b,
            "l3": l3, "le": le, "b0d": b0d, "b1d": b1d, "b2d": b2d,
        })
    return in_maps


def kernel(grid_input, dt, W0, b0, W1, b1, W2, b2, W3, b3, _run_kwargs=None):
    global _compiled
    grid_input = np.asarray(grid_input)
    if _compiled is None:
        _compiled = _build_module()
    nc = _compiled
    in_maps = _prepare_core_inputs(grid_input, np.asarray(dt),
                                   np.asarray(W0), np.asarray(b0),
                                   np.asarray(W1), np.asarray(b1),
                                   np.asarray(W2), np.asarray(b2),
                                   np.asarray(W3), np.asarray(b3))
    kw = _run_kwargs or {}
    r = run_bass_kernel_spmd(nc, in_maps, list(range(NCORES)), **kw)
    out = np.empty((B, 1, NT, NX), np.float32)
    out[:, 0, 0, :] = grid_input[:, 0, 0, :]
    for c in range(NCORES):
        out[BPC * c:BPC * (c + 1), 0, 1:, :] = r.results[c]["out"]
    kernel.last_results = r
    return out
